# revision 22
# baseline (speedup 1.0000x reference)
"""MoE layer (dense routing, 8 experts) on 8 TRN2 NeuronCores.

Expert-parallel: core e owns expert e (W1[e], b1[e], W2[e], b2[e]); x and the
gating network are replicated.  Each core computes
    y_e = gate[:, e] * (relu(x @ W1[e] + b1[e]) @ W2[e] + b2[e])
and the host sums the 8 per-core outputs.

Device layout (per core):
  - GEMM1 runs "transposed": hT[n, b] = sum_d W1[d, n] * xT[d, b], so both
    operands come straight from HBM in natural layout (x is pre-transposed on
    the host, once, for all cores).  b1 becomes a per-partition bias fused in
    the ReLU activation; the gate is folded into h (g > 0 so
    relu(z)*g == relu(z*g) is not even needed: we scale after the relu).
  - GEMM2 is normal orientation: y[b, o] = sum_h hT[h, b] * W2[h, o] with the
    hT 128-column slices as stationary operands.  b2 rides along as a 33rd
    contraction slice: an extra h-row holding gate[b] against an extra W2 row
    holding b2[o], which adds g[b]*b2[o] into the same PSUM accumulation.
  - The gate softmax for ALL 8 experts is computed on every core from
    replicated Wg/bg: logitsT[e, b] on 8 partitions, exp via ScalarE with the
    bg bias fused, then partition-dim sums/broadcasts via two matmuls against
    an all-ones / one-hot-row stationary matrix (zero-padded exp rows make the
    contraction a full K=128).

All matmuls are bf16 x bf16 -> fp32 PSUM.  Weights live in SBUF for the whole
kernel; x streams through in 512-column batch tiles.
"""

import numpy as np
import ml_dtypes

import concourse.bacc as bacc
import concourse.mybir as mybir
import concourse.tile as tile
from concourse.bass_utils import run_bass_kernel_spmd

B, D_IN, D_HID, D_OUT, E = 8192, 1024, 4096, 1024, 8
NCORES = 8
BT = 512                 # batch tile (matmul moving free dim)
P = 128
KD = D_IN // P           # 8 contraction subtiles for GEMM1
NH = D_HID // P          # 32 hidden tiles
NO = D_OUT // BT         # 2 output column tiles
MSUB = BT // P           # 4 output row subtiles per batch tile

BF16 = mybir.dt.bfloat16
F32 = mybir.dt.float32
AF = mybir.ActivationFunctionType

nbf16 = ml_dtypes.bfloat16


def build_nc(batch=B, passes=1):
    assert batch % BT == 0
    nb = batch // BT

    nc = bacc.Bacc(trn_type="TRN2")

    xt_d = nc.dram_tensor("xt", [D_IN, batch], BF16, kind="ExternalInput")
    w1_d = nc.dram_tensor("w1", [D_IN, D_HID], BF16, kind="ExternalInput")
    b1_d = nc.dram_tensor("b1c", [P, NH], F32, kind="ExternalInput")
    w2_d = nc.dram_tensor("w2", [D_HID, D_OUT], BF16, kind="ExternalInput")
    b2_d = nc.dram_tensor("b2r", [1, D_OUT], BF16, kind="ExternalInput")
    wg_d = nc.dram_tensor("wg", [KD, P, E], BF16, kind="ExternalInput")
    bg_d = nc.dram_tensor("bgc", [E, 1], F32, kind="ExternalInput")
    ones_d = nc.dram_tensor("ones", [P, P], BF16, kind="ExternalInput")
    sel_d = nc.dram_tensor("sel", [P, P], BF16, kind="ExternalInput")
    id_d = nc.dram_tensor("ident", [P, P], BF16, kind="ExternalInput")
    y_d = nc.dram_tensor("y", [batch, D_OUT], F32, kind="ExternalOutput")

    with tile.TileContext(nc) as tc:
        with (
            tc.tile_pool(name="const", bufs=1) as const,
            tc.tile_pool(name="wpool", bufs=1) as wpool,
            tc.tile_pool(name="xpool", bufs=8) as xpool,
            tc.tile_pool(name="hpool", bufs=33) as hpool,
            tc.tile_pool(name="gpool", bufs=2) as gpool,
            tc.tile_pool(name="ypool", bufs=3) as ypool,
            tc.tile_pool(name="gppool", bufs=8) as gppool,
            tc.tile_pool(name="ph", bufs=2, space="PSUM") as ph,
            tc.tile_pool(name="py", bufs=4, space="PSUM") as py,
            tc.tile_pool(name="pg", bufs=2, space="PSUM") as pg,
        ):
            # ---- persistent tiles -------------------------------------
            wg_sb = const.tile([P, KD, E], BF16, tag="wg")
            nc.sync.dma_start(wg_sb[:], wg_d[:].rearrange("k p e -> p k e"))
            b1_sb = const.tile([P, NH], F32, tag="b1")
            nc.sync.dma_start(b1_sb[:], b1_d[:])
            bg_sb = const.tile([E, 1], F32, tag="bg")
            nc.sync.dma_start(bg_sb[:], bg_d[:])
            ones_sb = const.tile([P, P], BF16, tag="ones")
            nc.sync.dma_start(ones_sb[:], ones_d[:])
            sel_sb = const.tile([P, P], BF16, tag="sel")
            nc.sync.dma_start(sel_sb[:], sel_d[:])
            id_sb = const.tile([P, P], BF16, tag="ident")
            nc.sync.dma_start(id_sb[:], id_d[:])

            # First b-tile's x arrives before the bulk weight load so the PE
            # can start (gate + GEMM1) while w2 is still streaming in.
            xts_first = []
            for kd in range(KD):
                t = xpool.tile([P, BT], BF16, tag="xt", name=f"xt0_{kd}")
                nc.sync.dma_start(t[:], xt_d[kd * P:(kd + 1) * P, 0:BT])
                xts_first.append(t)

            # w1 DMAs split into column chunks, chunk-major, so the first
            # GEMM1 n-tiles become runnable after ~2MB instead of 8MB.
            w1_sb = [wpool.tile([P, D_HID], BF16, tag=f"w1_{kd}",
                                name=f"w1_{kd}")
                     for kd in range(KD)]
            W1C = 4
            for c in range(W1C):
                cs = slice(c * (D_HID // W1C), (c + 1) * (D_HID // W1C))
                for kd in range(KD):
                    nc.sync.dma_start(w1_sb[kd][:, cs],
                                      w1_d[kd * P:(kd + 1) * P, cs])
            w2_sb = []
            for kh in range(NH):
                t = wpool.tile([P, D_OUT], BF16, tag=f"w2_{kh}")
                nc.sync.dma_start(t[:], w2_d[kh * P:(kh + 1) * P, :])
                w2_sb.append(t)
            # b2 broadcast to all partitions once, via a ones-matmul against a
            # zero-padded single-row staging tile.
            w2x = const.tile([P, D_OUT], BF16, tag="w2x")
            nc.vector.memset(w2x[:], 0.0)
            nc.sync.dma_start(w2x[0:1, :], b2_d[:])
            b2bc = const.tile([P, D_OUT], F32, tag="b2bc")
            for ot in range(NO):
                pb2 = pg.tile([P, BT], F32, tag="g", name=f"pb2_{ot}")
                nc.tensor.matmul(pb2[:], lhsT=ones_sb[:],
                                 rhs=w2x[:, ot * BT:(ot + 1) * BT],
                                 start=True, stop=True)
                nc.any.tensor_copy(out=b2bc[:, ot * BT:(ot + 1) * BT],
                                   in_=pb2[:])

            # exp of gate logits, zero-padded to full 128 partitions so the
            # broadcast matmuls contract over K=128
            exp_sb = const.tile([P, BT], BF16, tag="exp")
            nc.vector.memset(exp_sb[:], 0.0)

            # ---- main loop over batch tiles ---------------------------
            # passes>1 repeats the whole loop (same output) — used only by
            # the perf harness to measure device time as a wall-clock slope.
            for it, bt in enumerate(
                    [i for _ in range(passes) for i in range(nb)]):
                b0 = bt * BT

                if it == 0:
                    xts = xts_first
                else:
                    xts = []
                    for kd in range(KD):
                        t = xpool.tile([P, BT], BF16, tag="xt")
                        nc.sync.dma_start(
                            t[:], xt_d[kd * P:(kd + 1) * P, b0:b0 + BT])
                        xts.append(t)

                # gate: logitsT[e, b] for all 8 experts
                lg = pg.tile([E, BT], F32, tag="g")
                for kd in range(KD):
                    nc.tensor.matmul(
                        lg[:], lhsT=wg_sb[:, kd, :], rhs=xts[kd][:],
                        start=(kd == 0), stop=(kd == KD - 1))
                nc.scalar.activation(exp_sb[0:E, :], lg[:], AF.Exp,
                                     bias=bg_sb[:], scale=1.0)
                den = pg.tile([P, BT], F32, tag="g")
                nc.tensor.matmul(den[:], lhsT=ones_sb[:], rhs=exp_sb[:],
                                 start=True, stop=True)
                num = pg.tile([P, BT], F32, tag="g")
                nc.tensor.matmul(num[:], lhsT=sel_sb[:], rhs=exp_sb[:],
                                 start=True, stop=True)
                rec = gpool.tile([P, BT], F32, tag="rec")
                nc.vector.reciprocal(rec[:], den[:])
                gbc = gpool.tile([P, BT], BF16, tag="gbc")
                nc.vector.tensor_mul(out=gbc[:], in0=num[:], in1=rec[:])
                # per-partition gate columns for the GEMM2 epilogue: transpose
                # each 128-wide slice of the (partition-replicated) gbc and
                # keep column 0
                gps = []
                for ms in range(MSUB):
                    tp = pg.tile([P, P], BF16, tag="g", name=f"tp{ms}")
                    nc.tensor.transpose(
                        tp[:], gbc[:, ms * P:(ms + 1) * P], id_sb[:])
                    gp = gppool.tile([P, 1], F32, tag="gp", name=f"gp{ms}")
                    nc.any.tensor_copy(out=gp[:], in_=tp[:, 0:1])
                    gps.append(gp)

                # GEMM1: hT[n, b] = relu(sum_d W1[d,n] xT[d,b] + b1[n]) * g[b]
                hs = []
                for nt in range(NH):
                    acc = ph.tile([P, BT], F32, tag="acc")
                    for kd in range(KD):
                        nc.tensor.matmul(
                            acc[:],
                            lhsT=w1_sb[kd][:, nt * P:(nt + 1) * P],
                            rhs=xts[kd][:],
                            start=(kd == 0), stop=(kd == KD - 1))
                    h = hpool.tile([P, BT], BF16, tag="h")
                    nc.scalar.activation(h[:], acc[:], AF.Relu,
                                         bias=b1_sb[:, nt:nt + 1], scale=1.0)
                    nc.vector.tensor_mul(out=h[:], in0=h[:], in1=gbc[:])
                    hs.append(h)

                # GEMM2: y[b, o] = sum_h hT[h, b] W2[h, o], then the epilogue
                # fuses psum evacuation with the gated bias:
                #   y_sbuf = b2_bcast * g[b] + y_psum.
                # ot is the inner loop so each stationary h-slice load feeds
                # both output-column matmuls.
                for ms in range(MSUB):
                    accs = [py.tile([P, BT], F32, tag="acc", name=f"acc{ot}")
                            for ot in range(NO)]
                    for kh in range(NH):
                        lhsT = hs[kh][:, ms * P:(ms + 1) * P]
                        for ot in range(NO):
                            nc.tensor.matmul(
                                accs[ot][:],
                                lhsT=lhsT,
                                rhs=w2_sb[kh][:, ot * BT:(ot + 1) * BT],
                                start=(kh == 0), stop=(kh == NH - 1))
                    for ot in range(NO):
                        yt = ypool.tile([P, BT], F32, tag="y")
                        nc.vector.scalar_tensor_tensor(
                            out=yt[:],
                            in0=b2bc[:, ot * BT:(ot + 1) * BT],
                            scalar=gps[ms][:],
                            in1=accs[ot][:],
                            op0=mybir.AluOpType.mult,
                            op1=mybir.AluOpType.add)
                        nc.sync.dma_start(
                            y_d[b0 + ms * P:b0 + (ms + 1) * P,
                                ot * BT:(ot + 1) * BT],
                            yt[:])

    nc.finalize()
    return nc


def make_in_maps(x, W1, b1, W2, b2, Wg, bg, batch=B):
    """Host-side sharding prep: transpose x once, cast matmul operands to
    bf16, reshape biases to the on-chip layouts."""
    f32 = np.float32
    xt = np.ascontiguousarray(x.astype(f32).T).astype(nbf16)      # [D_IN, B]
    wg = np.ascontiguousarray(
        Wg.astype(f32).reshape(KD, P, E)).astype(nbf16)
    bgc = np.ascontiguousarray(bg.astype(f32).reshape(E, 1))
    ones = np.ones((P, P), dtype=nbf16)
    ident = np.eye(P, dtype=nbf16)

    in_maps = []
    for e in range(NCORES):
        sel = np.zeros((P, P), dtype=nbf16)
        sel[e, :] = 1.0
        in_maps.append({
            "xt": xt,
            "w1": np.ascontiguousarray(W1[e].astype(f32)).astype(nbf16),
            "b1c": np.ascontiguousarray(
                b1[e].astype(f32).reshape(NH, P).T),
            "w2": np.ascontiguousarray(W2[e].astype(f32)).astype(nbf16),
            "b2r": np.ascontiguousarray(
                b2[e].astype(f32).reshape(1, D_OUT)).astype(nbf16),
            "wg": wg,
            "bgc": bgc,
            "ones": ones,
            "sel": sel,
            "ident": ident,
        })
    return in_maps


def kernel(x, W1, b1, W2, b2, Wg, bg):
    in_maps = make_in_maps(x, W1, b1, W2, b2, Wg, bg)
    nc = build_nc(B)
    res = run_bass_kernel_spmd(nc, in_maps, core_ids=list(range(NCORES)))
    out = res.results[0]["y"].astype(np.float64)
    for e in range(1, NCORES):
        out += res.results[e]["y"]
    return out.astype(np.float32)
